# revision 19
# baseline (speedup 1.0000x reference)
"""ConnectivityLoss3D on 8 Trainium2 NeuronCores.

loss = sum_b[ is_gt_b * mean(softplus(x) - x*y) + 1e-4 * mean(box27(pred)==pred) ]
with pred = (x>0) in {0,1}, box27 = periodic 3x3x3 neighbor sum.
box27(pred)==pred  <=>  u := box27 - pred == 0 (no positive 26-neighbor);
count elem = relu(1 - u)  (u is an integer >= 0).

Sharding: core c owns d-slices [24c, 24c+24). Host re-lays each core's slab as
[row=(b*192+w), j=(d halo 26), h] so partitions pack 768 rows = 6 tiles of 128.

v2 pipeline (DMA-bound target ~90us):
  - x,y cast f32->bf16 during DMA (SWDGE/gpsimd), one ~2.5MB DMA per tile,
    all 12 issued up-front; deep pools keep SDMA streaming.
  - DVE: pred = is_gt(x) one op (bf16 4x), d-axis box sum s2 (2 adds, bf16 2x,
    196-wide halo'd layout keeps writes pair-aligned), x*y via one fused
    STT+accum (bf16), 2 tiny h-wrap halo copies.
  - ACT: softplus(x) single pass (F.Softplus) with fused row-sum accum;
    count relu(1-psum) from PSUM per chunk.
  - TensorE: per 8-j chunk: 12 banded tw matmuls (w-sum x 3 h-shifts), 4 m3
    matmuls on pre-shifted cross rows (w-wrap), 4 -I matmuls subtracting pred.
  - Cross rows gathered with ONE SBUF->SBUF DMA per source row using a
    3-shift overlapping access pattern (sync/HWDGE queue).
All per-row partial sums accumulate into a [128, 30] stats tile, combined on
host in fp64.
"""
import os
import sys

sys.path.insert(0, "/opt/trn_rl_repo")

import numpy as np
import ml_dtypes

import concourse.bass as bass
import concourse.mybir as mybir
from concourse.tile import TileContext
from concourse.bass_utils import run_bass_kernel_spmd
from concourse.vector_clock import ScopedClock
from concourse.tile_rust import add_dep_helper

# ---------------------------------------------------------------- constants
B, D, H, Wd = 4, 192, 192, 192
NCORES = 8
DSH = D // NCORES          # 24 own d-slices per core
J = DSH + 2                # with +-1 halo
P = 128
ROWS = B * Wd              # 768 partition rows = 6 tiles of 128
NT = ROWS // P             # 6
VOX = D * H * Wd           # voxels per sample
W_CONN = 1e-4

F = mybir.ActivationFunctionType
A = mybir.AluOpType

# stats column layout: per tile 5 cols: xy sp cnt0 cnt1 cnt2
TCOLS = 5
NSTAT = NT * TCOLS  # 30


def _patched_drain_and_barrier(self, tick_clock, wait_clock):
    # Walrus encodes at most one sem-wait per instruction; the stock Tile exit
    # drain aggregates one wait per busy processor. Split across drains.
    nc = self.nc
    drain_inst = nc.sync.drain()
    wait_clock.add_sem_waits(
        drain_inst.ins, ScopedClock({None: tick_clock.global_clock})
    )
    si = drain_inst.ins.sync_info
    if si is not None and len(si.on_wait) > 1:
        waits = list(si.on_wait)
        drain_inst.ins.sync_info = mybir.SyncInfo(
            on_wait=waits[:1], on_update=list(si.on_update)
        )
        for w in waits[1:]:
            d2 = nc.sync.drain()
            d2.ins.sync_info = mybir.SyncInfo(on_wait=[w], on_update=[])
    nc.all_engine_barrier()
    popped = nc._tile_sem_poison_stack.pop()
    assert popped is self._sem_poison
    nc.clear_and_free_semaphores(list(self.sems.allocated().values()))
    nc.all_engine_barrier()


TileContext._drain_and_barrier = _patched_drain_and_barrier

_ORIG_LOWER = TileContext._lower_ordered_insts


def _patched_lower_ordered_insts(self, ordered):
    # Split multi-wait instructions: walrus encodes one sem-wait per
    # instruction, so park extra waits on same-engine NoOps just before.
    for bb in list(ordered.keys()):
        insts = ordered[bb]
        out = []
        for inst in insts:
            si = getattr(inst, "sync_info", None)
            waits = list(si.on_wait) if si is not None else []
            if len(waits) > 1:
                for k, w in enumerate(waits[:-1]):
                    nop = mybir.InstNoOp(name=f"{inst.name}_wsplit{k}",
                                         ins=[], outs=[])
                    nop.engine = inst.engine
                    nop.sync_info = mybir.SyncInfo(on_wait=[w], on_update=[])
                    out.append(nop)
                inst.sync_info = mybir.SyncInfo(
                    on_wait=[waits[-1]], on_update=list(si.on_update))
            out.append(inst)
        ordered[bb] = out
    return _ORIG_LOWER(self, ordered)


TileContext._lower_ordered_insts = _patched_lower_ordered_insts


def _build_consts():
    """Within-tile w-adjacency matrices, cross-row matrices, gather lists."""
    tw = np.zeros((NT, P, P), np.float32)
    m3 = np.zeros((NT, 12, P), np.float32)
    gather = []  # per tile: list of src global rows (slot order)
    for t in range(NT):
        cross_pairs = []
        for ml in range(P):
            R = P * t + ml
            b, w = R // Wd, R % Wd
            for dw in (-1, 0, 1):
                Rs = b * Wd + (w + dw) % Wd
                if P * t <= Rs < P * (t + 1):
                    tw[t, Rs - P * t, ml] = 1.0
                else:
                    cross_pairs.append((Rs, ml))
        srcs = sorted({s for s, _ in cross_pairs})
        assert len(srcs) <= 4, srcs
        slot = {s: i for i, s in enumerate(srcs)}
        for s, ml in cross_pairs:
            for k in range(3):
                m3[t, 3 * slot[s] + k, ml] = 1.0
        gather.append(srcs)
    return tw, m3, gather


_TW, _M3, _GATHER = _build_consts()


def _pack_consts():
    c = np.zeros((P, 2 * NT * P + P), np.float32)  # [128, 1664]
    for t in range(NT):
        c[:, t * P:(t + 1) * P] = _TW[t]
        c[0:12, NT * P + t * P: NT * P + (t + 1) * P] = _M3[t]
    c[:, 2 * NT * P:] = -np.eye(P, dtype=np.float32)
    return c.astype(ml_dtypes.bfloat16)


_CONSTS = _pack_consts()


def _build_nc():
    nc = bass.Bass(trn_type="TRN2")
    x_d = nc.dram_tensor("x", [ROWS, J, H], mybir.dt.float32, kind="ExternalInput")
    y_d = nc.dram_tensor("y", [ROWS, DSH, H], mybir.dt.float32, kind="ExternalInput")
    c_d = nc.dram_tensor("consts", [P, 2 * NT * P + P], mybir.dt.bfloat16,
                         kind="ExternalInput")
    st_d = nc.dram_tensor("stats", [P, NSTAT], mybir.dt.float32,
                          kind="ExternalOutput")

    with TileContext(nc) as tc:
        with tc.tile_pool(name="xp", bufs=3) as xp, \
             tc.tile_pool(name="yp", bufs=3) as yp, \
             tc.tile_pool(name="predp", bufs=3) as predp, \
             tc.tile_pool(name="s2p", bufs=3) as s2p, \
             tc.tile_pool(name="ep", bufs=2) as ep, \
             tc.tile_pool(name="scrp", bufs=2) as scrp, \
             tc.tile_pool(name="crossp", bufs=3) as crossp, \
             tc.tile_pool(name="singles", bufs=1) as singles, \
             tc.tile_pool(name="psump", bufs=2, space="PSUM") as psump:

            consts = singles.tile([P, 2 * NT * P + P], mybir.dt.bfloat16)
            nc.sync.dma_start(out=consts, in_=c_d[:, :])
            stats = singles.tile([P, NSTAT], mybir.dt.float32)

            def tw_ap(t):
                return consts[:, t * P:(t + 1) * P]

            def m3_ap(t):
                return consts[0:12, NT * P + t * P: NT * P + (t + 1) * P]

            def negI_ap():
                return consts[:, 2 * NT * P:]

            # ---- input DMAs: cast f32->bf16 (SWDGE/gpsimd). 3 tiles deep,
            # issued ahead of use; pool rotation inserts the waits.
            x_tiles = {}
            y_tiles = {}

            def issue_loads(t):
                if t >= NT:
                    return
                r0 = P * t
                xt = xp.tile([P, J, H], mybir.dt.bfloat16, tag="x")
                nc.gpsimd.dma_start(out=xt, in_=x_d[r0:r0 + P, :, :])
                x_tiles[t] = xt
                yt = yp.tile([P, DSH, H], mybir.dt.bfloat16, tag="y")
                nc.gpsimd.dma_start(out=yt, in_=y_d[r0:r0 + P, :, :])
                y_tiles[t] = yt

            for t in range(3):
                issue_loads(t)

            s2_tiles = {}
            pred_tiles = {}
            cross_tiles = {}

            def front_a(t):
                """pred + d-sum (DVE) for tile t."""
                xt = x_tiles[t]
                pred = predp.tile([P, J, H], mybir.dt.bfloat16, tag="pred")
                nc.vector.tensor_scalar(pred, xt, 0.0, None, A.is_gt)
                pred_tiles[t] = pred

                s2 = s2p.tile([P, DSH, 196], mybir.dt.bfloat16, tag="s2")
                nc.vector.tensor_tensor(
                    s2[:, :, 2:194], pred[:, 0:DSH, :], pred[:, 1:DSH + 1, :], A.add)
                nc.vector.tensor_tensor(
                    s2[:, :, 2:194], s2[:, :, 2:194], pred[:, 2:DSH + 2, :], A.add)
                nc.vector.tensor_copy(s2[:, :, 1:2], s2[:, :, 193:194])
                nc.vector.tensor_copy(s2[:, :, 194:195], s2[:, :, 2:3])
                s2_tiles[t] = s2

            def front_xy(t):
                """x*y via fused STT+accum (bf16)."""
                xt, yt = x_tiles[t], y_tiles[t]
                scr = scrp.tile([P, DSH, H], mybir.dt.bfloat16, tag="sxy")
                nc.vector.scalar_tensor_tensor(
                    out=scr, in0=xt[:, 1:DSH + 1, :],
                    scalar=0.0, in1=yt, op0=A.bypass, op1=A.mult,
                    accum_out=stats[:, t * TCOLS: t * TCOLS + 1])

            def front_exp(t):
                """e = exp(x) on ACT (bf16 out; softplus = ln(1+e) next)."""
                xt = x_tiles[t]
                e = ep.tile([P, DSH, H], mybir.dt.bfloat16, tag="e")
                nc.scalar.activation(e, xt[:, 1:DSH + 1, :], F.Exp)
                return e

            def front_ln(t, e):
                """softplus = ln(1+e) on ACT with fused row-sum accum."""
                scr = scrp.tile([P, DSH, H], mybir.dt.bfloat16, tag="ssp")
                nc.scalar.activation(
                    scr, e, F.Ln, bias=1.0,
                    accum_out=stats[:, t * TCOLS + 1: t * TCOLS + 2])

            def gather(t, early):
                """Cross-row gathers for tile t (sync/HWDGE queue; issue is
                cheap there and SBUF->SBUF doesn't consume HBM bandwidth).
                early=True: sources from tiles <= t; else from tile t+1.
                First use of a pool buffer pads the source list to 4 (dup of
                src 0, m3 weight 0) so the m3 matmul never reads NaN garbage
                (0 * NaN = NaN). Later reuses inherit finite data."""
                srcs = list(_GATHER[t])
                if early:
                    cross = crossp.tile([12, DSH, H], mybir.dt.bfloat16,
                                        tag="cross")
                    cross_tiles[t] = cross
                if t < 3:
                    srcs += [srcs[0]] * (4 - len(srcs))
                cross = cross_tiles[t]
                for si_, src in enumerate(srcs):
                    ts, pl = src // P, src % P
                    if (ts <= t) != early:
                        continue
                    for k in range(3):
                        nc.sync.dma_start(
                            out=cross[3 * si_ + k: 3 * si_ + k + 1, :, :],
                            in_=s2_tiles[ts][pl:pl + 1, :, 1 + k:193 + k])

            def back_chunk(t, c, count_on_dve=False):
                """PE chunk c + count for tile t. Count u==0 runs on ACT
                (relu(1-u)) or DVE (is_equal, to balance engine load)."""
                s2 = s2_tiles[t]
                pred = pred_tiles[t]
                cross = cross_tiles[t]
                jbase = 8 * c
                psum = psump.tile([P, 8, 256], mybir.dt.float32, tag="psum")
                mms = []
                # order: tw, negI (inputs ready at iteration start), m3 last
                # (needs this iteration's late cross gathers — max slack).
                for dh_i, dh in enumerate((-1, 0, 1)):
                    for jc in range(4):
                        js = 2 * jc
                        mms.append(nc.tensor.matmul(
                            psum[:, js:js + 2, 0:192], tw_ap(t),
                            s2[:, jbase + js:jbase + js + 2,
                               2 + dh:194 + dh],
                            start=(dh_i == 0), stop=False))
                for jc in range(4):
                    js = 2 * jc
                    mms.append(nc.tensor.matmul(
                        psum[:, js:js + 2, 0:192], negI_ap(),
                        pred[:, 1 + jbase + js:3 + jbase + js, :],
                        start=False, stop=False))
                for jc in range(4):
                    js = 2 * jc
                    mms.append(nc.tensor.matmul(
                        psum[:, js:js + 2, 0:192], m3_ap(t),
                        cross[:, jbase + js:jbase + js + 2, :],
                        start=False, stop=(jc == 3)))
                # psum = box27 - pred = u >= 0; count elem = 1[u == 0]
                scr = scrp.tile([P, 8, H], mybir.dt.bfloat16, tag="scnt")
                acc = stats[:, t * TCOLS + 2 + c: t * TCOLS + 3 + c]
                if count_on_dve:
                    cntop = nc.vector.tensor_scalar(
                        scr, psum[:, :, 0:192], 0.0, 0.0, A.is_equal,
                        A.add, accum_out=acc)
                else:
                    cntop = nc.scalar.activation(
                        scr, psum[:, :, 0:192], F.Relu,
                        bias=1.0, scale=-1.0, accum_out=acc)
                for mm in mms:
                    add_dep_helper(cntop.ins, mm.ins,
                                   reason="count reads whole psum chunk")

            front_a(0)
            gather(0, early=True)
            e0 = front_exp(0)
            front_ln(0, e0)
            front_xy(0)
            issue_loads(3)
            for t in range(1, NT):
                front_a(t)
                gather(t, early=True)       # srcs in tiles <= t
                gather(t - 1, early=False)  # srcs in tile t (just computed)
                e = front_exp(t)
                front_ln(t, e)
                front_xy(t)
                back_chunk(t - 1, 0)
                back_chunk(t - 1, 1)
                back_chunk(t - 1, 2)
                issue_loads(t + 3)
            for c in range(3):
                back_chunk(NT - 1, c)

            nc.sync.dma_start(out=st_d[:, :], in_=stats)
    return nc


_NC_CACHE = None
LAST_RESULTS = None


def _ensure_ntff_hook():
    """Register the axon NTFF profiling hook (absent from this image's
    antenv) so trace=True can measure HW exec time. Trace-path only."""
    import contextlib
    import ctypes
    import types

    try:
        from antenv.axon_hooks import get_axon_ntff_profile_hook  # noqa: F401
        return
    except ImportError:
        pass

    holder = {}
    mod = types.ModuleType("antenv.axon_hooks")
    mod.set_axon_ntff_profile_hook = lambda h: holder.__setitem__("h", h)
    mod.get_axon_ntff_profile_hook = lambda: holder.get("h")
    sys.modules["antenv.axon_hooks"] = mod
    import antenv

    antenv.axon_hooks = mod

    so_path = "/opt/axon/libaxon_pjrt.so"
    try:
        lib = ctypes.CDLL(so_path)
    except OSError:
        return
    if not hasattr(lib, "axon_start_nrt_profile"):
        return
    lib.axon_start_nrt_profile.argtypes = [
        ctypes.POINTER(ctypes.c_int64), ctypes.c_size_t]
    lib.axon_start_nrt_profile.restype = ctypes.c_int64
    lib.axon_stop_nrt_profile.argtypes = [ctypes.c_char_p]
    lib.axon_stop_nrt_profile.restype = ctypes.c_int64

    @contextlib.contextmanager
    def _hook(output_dir, device_ids):
        import jax

        jax.devices()
        if device_ids:
            ids = (ctypes.c_int64 * len(device_ids))(*device_ids)
            rc = lib.axon_start_nrt_profile(ids, len(device_ids))
        else:
            rc = lib.axon_start_nrt_profile(None, 0)
        if rc != 0:
            raise RuntimeError(f"axon_start_nrt_profile rc={rc}")
        try:
            yield
        finally:
            n = lib.axon_stop_nrt_profile(str(output_dir).encode())
            print(f"profile: {n} file(s) written to {output_dir}",
                  file=sys.stderr)

    mod.set_axon_ntff_profile_hook(_hook)

    # keep trace artifacts local — no bucket in this container
    import concourse.bass_utils as bu

    bu.upload_artifacts = lambda tmpdir: tmpdir


def _get_nc():
    global _NC_CACHE
    if _NC_CACHE is None:
        _NC_CACHE = _build_nc()
    return _NC_CACHE


def _shard_inputs(x, y):
    """Host re-layout: per-core [row=(b*192+w), j, h] arrays."""
    xs, ys = [], []
    x5 = x[:, 0]  # [B, D, H, W]
    y5 = y[:, 0]
    for c in range(NCORES):
        dids = (np.arange(DSH * c - 1, DSH * (c + 1) + 1)) % D          # 26
        # [B, 26, H, W] -> [B, W, 26, H] -> [768, 26, 192]
        xs.append(np.ascontiguousarray(
            x5[:, dids].transpose(0, 3, 1, 2)).reshape(ROWS, J, H))
        ys.append(np.ascontiguousarray(
            y5[:, DSH * c:DSH * (c + 1)].transpose(0, 3, 1, 2)
        ).reshape(ROWS, DSH, H))
    return xs, ys


def kernel(x, y, is_gt):
    global LAST_RESULTS
    x = np.asarray(x, dtype=np.float32)
    y = np.asarray(y, dtype=np.float32)
    is_gt = np.asarray(is_gt, dtype=np.float32)

    nc = _get_nc()
    xs, ys = _shard_inputs(x, y)
    in_maps = [{"x": xs[c], "y": ys[c], "consts": _CONSTS}
               for c in range(NCORES)]
    trace = bool(os.environ.get("KERNEL_TRACE"))
    if trace:
        _ensure_ntff_hook()
    res = run_bass_kernel_spmd(nc, in_maps, core_ids=list(range(NCORES)),
                               trace=trace)
    LAST_RESULTS = res

    # host-side final reduction in fp64
    b_of = (np.arange(ROWS) // Wd)                       # global row -> sample
    S_xy = np.zeros(B)
    S_sp = np.zeros(B)
    S_cnt = np.zeros(B)
    for c in range(NCORES):
        st = res.results[c]["stats"].astype(np.float64)  # [128, NSTAT]
        for col in range(NSTAT):
            t, k = col // TCOLS, col % TCOLS
            tgt = S_xy if k < 1 else (S_sp if k < 2 else S_cnt)
            np.add.at(tgt, b_of[P * t:P * (t + 1)], st[:, col])
    bce = (S_sp - S_xy) / VOX
    conn = S_cnt / VOX
    loss = np.sum(is_gt.astype(np.float64) * bce + W_CONN * conn)
    return np.array([loss], dtype=np.float32)


# revision 23
# speedup vs baseline: 1.0281x; 1.0281x over previous
"""ConnectivityLoss3D on 8 Trainium2 NeuronCores.

loss = sum_b[ is_gt_b * mean(softplus(x) - x*y) + 1e-4 * mean(box27(pred)==pred) ]
with pred = (x>0) in {0,1}, box27 = periodic 3x3x3 neighbor sum.
box27(pred)==pred  <=>  u := box27 - pred == 0 (no positive 26-neighbor);
count elem = relu(1 - u)  (u is an integer >= 0).

Sharding: core c owns d-slices [24c, 24c+24). Host re-lays each core's slab as
[row=(b*192+w), j=(d halo 26), h] so partitions pack 768 rows = 6 tiles of 128.

v2 pipeline (DMA-bound target ~90us):
  - x,y cast f32->bf16 during DMA (SWDGE/gpsimd), one ~2.5MB DMA per tile,
    all 12 issued up-front; deep pools keep SDMA streaming.
  - DVE: pred = is_gt(x) one op (bf16 4x), d-axis box sum s2 (2 adds, bf16 2x,
    196-wide halo'd layout keeps writes pair-aligned), x*y via one fused
    STT+accum (bf16), 2 tiny h-wrap halo copies.
  - ACT: softplus(x) single pass (F.Softplus) with fused row-sum accum;
    count relu(1-psum) from PSUM per chunk.
  - TensorE: per 8-j chunk: 12 banded tw matmuls (w-sum x 3 h-shifts), 4 m3
    matmuls on pre-shifted cross rows (w-wrap), 4 -I matmuls subtracting pred.
  - Cross rows gathered with ONE SBUF->SBUF DMA per source row using a
    3-shift overlapping access pattern (sync/HWDGE queue).
All per-row partial sums accumulate into a [128, 30] stats tile, combined on
host in fp64.
"""
import os
import sys

sys.path.insert(0, "/opt/trn_rl_repo")

import numpy as np
import ml_dtypes

import concourse.bass as bass
import concourse.mybir as mybir
from concourse.tile import TileContext
from concourse.bass_utils import run_bass_kernel_spmd
from concourse.vector_clock import ScopedClock
from concourse.tile_rust import add_dep_helper

# ---------------------------------------------------------------- constants
B, D, H, Wd = 4, 192, 192, 192
NCORES = 8
DSH = D // NCORES          # 24 own d-slices per core
J = DSH + 2                # with +-1 halo
P = 128
ROWS = B * Wd              # 768 partition rows = 6 tiles of 128
NT = ROWS // P             # 6
VOX = D * H * Wd           # voxels per sample
W_CONN = 1e-4

F = mybir.ActivationFunctionType
A = mybir.AluOpType

# stats column layout: per tile 5 cols: xy sp cnt0 cnt1 cnt2
TCOLS = 5
NSTAT = NT * TCOLS  # 30


def _patched_drain_and_barrier(self, tick_clock, wait_clock):
    # Walrus encodes at most one sem-wait per instruction; the stock Tile exit
    # drain aggregates one wait per busy processor. Split across drains.
    nc = self.nc
    drain_inst = nc.sync.drain()
    wait_clock.add_sem_waits(
        drain_inst.ins, ScopedClock({None: tick_clock.global_clock})
    )
    si = drain_inst.ins.sync_info
    if si is not None and len(si.on_wait) > 1:
        waits = list(si.on_wait)
        drain_inst.ins.sync_info = mybir.SyncInfo(
            on_wait=waits[:1], on_update=list(si.on_update)
        )
        for w in waits[1:]:
            d2 = nc.sync.drain()
            d2.ins.sync_info = mybir.SyncInfo(on_wait=[w], on_update=[])
    nc.all_engine_barrier()
    popped = nc._tile_sem_poison_stack.pop()
    assert popped is self._sem_poison
    nc.clear_and_free_semaphores(list(self.sems.allocated().values()))
    nc.all_engine_barrier()


TileContext._drain_and_barrier = _patched_drain_and_barrier

_ORIG_LOWER = TileContext._lower_ordered_insts


def _patched_lower_ordered_insts(self, ordered):
    # Split multi-wait instructions: walrus encodes one sem-wait per
    # instruction, so park extra waits on same-engine NoOps just before.
    for bb in list(ordered.keys()):
        insts = ordered[bb]
        out = []
        for inst in insts:
            si = getattr(inst, "sync_info", None)
            waits = list(si.on_wait) if si is not None else []
            if len(waits) > 1:
                for k, w in enumerate(waits[:-1]):
                    nop = mybir.InstNoOp(name=f"{inst.name}_wsplit{k}",
                                         ins=[], outs=[])
                    nop.engine = inst.engine
                    nop.sync_info = mybir.SyncInfo(on_wait=[w], on_update=[])
                    out.append(nop)
                inst.sync_info = mybir.SyncInfo(
                    on_wait=[waits[-1]], on_update=list(si.on_update))
            out.append(inst)
        ordered[bb] = out
    return _ORIG_LOWER(self, ordered)


TileContext._lower_ordered_insts = _patched_lower_ordered_insts


def _build_consts():
    """Within-tile w-adjacency matrices, cross-row matrices, gather lists."""
    tw = np.zeros((NT, P, P), np.float32)
    m3 = np.zeros((NT, 12, P), np.float32)
    gather = []  # per tile: list of src global rows (slot order)
    for t in range(NT):
        cross_pairs = []
        for ml in range(P):
            R = P * t + ml
            b, w = R // Wd, R % Wd
            for dw in (-1, 0, 1):
                Rs = b * Wd + (w + dw) % Wd
                if P * t <= Rs < P * (t + 1):
                    tw[t, Rs - P * t, ml] = 1.0
                else:
                    cross_pairs.append((Rs, ml))
        srcs = sorted({s for s, _ in cross_pairs})
        assert len(srcs) <= 4, srcs
        slot = {s: i for i, s in enumerate(srcs)}
        for s, ml in cross_pairs:
            for k in range(3):
                m3[t, 3 * slot[s] + k, ml] = 1.0
        gather.append(srcs)
    return tw, m3, gather


_TW, _M3, _GATHER = _build_consts()


def _pack_consts():
    c = np.zeros((P, 2 * NT * P + P), np.float32)  # [128, 1664]
    for t in range(NT):
        c[:, t * P:(t + 1) * P] = _TW[t]
        c[0:12, NT * P + t * P: NT * P + (t + 1) * P] = _M3[t]
    c[:, 2 * NT * P:] = -np.eye(P, dtype=np.float32)
    return c.astype(ml_dtypes.bfloat16)


_CONSTS = _pack_consts()


def _build_nc():
    nc = bass.Bass(trn_type="TRN2")
    x_d = nc.dram_tensor("x", [ROWS, J, H], mybir.dt.float32, kind="ExternalInput")
    y_d = nc.dram_tensor("y", [ROWS, DSH, H], mybir.dt.float32, kind="ExternalInput")
    c_d = nc.dram_tensor("consts", [P, 2 * NT * P + P], mybir.dt.bfloat16,
                         kind="ExternalInput")
    st_d = nc.dram_tensor("stats", [P, NSTAT], mybir.dt.float32,
                          kind="ExternalOutput")

    with TileContext(nc) as tc:
        with tc.tile_pool(name="xp", bufs=2) as xp, \
             tc.tile_pool(name="yp", bufs=2) as yp, \
             tc.tile_pool(name="predp", bufs=3) as predp, \
             tc.tile_pool(name="s2p", bufs=3) as s2p, \
             tc.tile_pool(name="ep", bufs=1) as ep, \
             tc.tile_pool(name="scrp", bufs=1) as scrp, \
             tc.tile_pool(name="scntp", bufs=2) as scntp, \
             tc.tile_pool(name="crossp", bufs=3) as crossp, \
             tc.tile_pool(name="singles", bufs=1) as singles, \
             tc.tile_pool(name="psump", bufs=2, space="PSUM") as psump:

            consts = singles.tile([P, 2 * NT * P + P], mybir.dt.bfloat16)
            nc.sync.dma_start(out=consts, in_=c_d[:, :])
            stats = singles.tile([P, NSTAT], mybir.dt.float32)

            def tw_ap(t):
                return consts[:, t * P:(t + 1) * P]

            def m3_ap(t):
                return consts[0:12, NT * P + t * P: NT * P + (t + 1) * P]

            def negI_ap():
                return consts[:, 2 * NT * P:]

            # ---- input DMAs: plain f32 on sync (HWDGE, full SDMA rate;
            # SWDGE cast-DMAs measured ~2x slower). 2 tiles deep.
            x_tiles = {}
            y_tiles = {}

            def issue_loads(t):
                if t >= NT:
                    return
                r0 = P * t
                xt = xp.tile([P, J, H], mybir.dt.float32, tag="x")
                nc.sync.dma_start(out=xt, in_=x_d[r0:r0 + P, :, :])
                x_tiles[t] = xt
                yt = yp.tile([P, DSH, H], mybir.dt.float32, tag="y")
                nc.sync.dma_start(out=yt, in_=y_d[r0:r0 + P, :, :])
                y_tiles[t] = yt

            for t in range(2):
                issue_loads(t)

            s2_tiles = {}
            pred_tiles = {}
            cross_tiles = {}

            def front_a(t):
                """pred + d-sum (DVE) for tile t."""
                xt = x_tiles[t]
                pred = predp.tile([P, J, H], mybir.dt.bfloat16, tag="pred")
                nc.vector.tensor_scalar(pred, xt, 0.0, None, A.is_gt)
                pred_tiles[t] = pred

                s2 = s2p.tile([P, DSH, 196], mybir.dt.bfloat16, tag="s2")
                nc.vector.tensor_tensor(
                    s2[:, :, 2:194], pred[:, 0:DSH, :], pred[:, 1:DSH + 1, :], A.add)
                nc.vector.tensor_tensor(
                    s2[:, :, 2:194], s2[:, :, 2:194], pred[:, 2:DSH + 2, :], A.add)
                nc.vector.tensor_copy(s2[:, :, 1:2], s2[:, :, 193:194])
                nc.vector.tensor_copy(s2[:, :, 194:195], s2[:, :, 2:3])
                s2_tiles[t] = s2

            def front_xy(t):
                """x*y via fused STT+accum (bf16)."""
                xt, yt = x_tiles[t], y_tiles[t]
                scr = scrp.tile([P, DSH, H], mybir.dt.bfloat16, tag="sxy")
                nc.vector.scalar_tensor_tensor(
                    out=scr, in0=xt[:, 1:DSH + 1, :],
                    scalar=0.0, in1=yt, op0=A.bypass, op1=A.mult,
                    accum_out=stats[:, t * TCOLS: t * TCOLS + 1])

            def front_exp(t):
                """e = exp(x) on ACT (bf16 out; softplus = ln(1+e) next)."""
                xt = x_tiles[t]
                e = ep.tile([P, DSH, H], mybir.dt.bfloat16, tag="e")
                nc.scalar.activation(e, xt[:, 1:DSH + 1, :], F.Exp)
                return e

            def front_ln(t, e):
                """softplus = ln(1+e) on ACT with fused row-sum accum."""
                scr = scrp.tile([P, DSH, H], mybir.dt.bfloat16, tag="ssp")
                nc.scalar.activation(
                    scr, e, F.Ln, bias=1.0,
                    accum_out=stats[:, t * TCOLS + 1: t * TCOLS + 2])

            def gather(t, early):
                """Cross-row gathers for tile t (sync/HWDGE queue; issue is
                cheap there and SBUF->SBUF doesn't consume HBM bandwidth).
                early=True: sources from tiles <= t; else from tile t+1.
                First use of a pool buffer pads the source list to 4 (dup of
                src 0, m3 weight 0) so the m3 matmul never reads NaN garbage
                (0 * NaN = NaN). Later reuses inherit finite data."""
                srcs = list(_GATHER[t])
                if early:
                    cross = crossp.tile([12, DSH, H], mybir.dt.bfloat16,
                                        tag="cross")
                    cross_tiles[t] = cross
                if t < 3:
                    srcs += [srcs[0]] * (4 - len(srcs))
                cross = cross_tiles[t]
                for si_, src in enumerate(srcs):
                    ts, pl = src // P, src % P
                    if (ts <= t) != early:
                        continue
                    for k in range(3):
                        nc.gpsimd.dma_start(
                            out=cross[3 * si_ + k: 3 * si_ + k + 1, :, :],
                            in_=s2_tiles[ts][pl:pl + 1, :, 1 + k:193 + k])

            def back_chunk(t, c, count_on_dve=False):
                """PE chunk c + count for tile t. Count u==0 runs on ACT
                (relu(1-u)) or DVE (is_equal, to balance engine load)."""
                s2 = s2_tiles[t]
                pred = pred_tiles[t]
                cross = cross_tiles[t]
                jbase = 8 * c
                psum = psump.tile([P, 8, 256], mybir.dt.float32, tag="psum")
                mms = []
                # order: tw, negI (inputs ready at iteration start), m3 last
                # (needs this iteration's late cross gathers — max slack).
                for dh_i, dh in enumerate((-1, 0, 1)):
                    for jc in range(4):
                        js = 2 * jc
                        mms.append(nc.tensor.matmul(
                            psum[:, js:js + 2, 0:192], tw_ap(t),
                            s2[:, jbase + js:jbase + js + 2,
                               2 + dh:194 + dh],
                            start=(dh_i == 0), stop=False))
                for jc in range(4):
                    js = 2 * jc
                    mms.append(nc.tensor.matmul(
                        psum[:, js:js + 2, 0:192], negI_ap(),
                        pred[:, 1 + jbase + js:3 + jbase + js, :],
                        start=False, stop=False))
                for jc in range(4):
                    js = 2 * jc
                    mms.append(nc.tensor.matmul(
                        psum[:, js:js + 2, 0:192], m3_ap(t),
                        cross[:, jbase + js:jbase + js + 2, :],
                        start=False, stop=(jc == 3)))
                # psum = box27 - pred = u >= 0; count elem = 1[u == 0]
                scr = scntp.tile([P, 8, H], mybir.dt.bfloat16, tag="scnt")
                acc = stats[:, t * TCOLS + 2 + c: t * TCOLS + 3 + c]
                if count_on_dve:
                    cntop = nc.vector.tensor_scalar(
                        scr, psum[:, :, 0:192], 0.0, 0.0, A.is_equal,
                        A.add, accum_out=acc)
                else:
                    cntop = nc.scalar.activation(
                        scr, psum[:, :, 0:192], F.Relu,
                        bias=1.0, scale=-1.0, accum_out=acc)
                for mm in mms:
                    add_dep_helper(cntop.ins, mm.ins,
                                   reason="count reads whole psum chunk")

            front_a(0)
            gather(0, early=True)
            e0 = front_exp(0)
            front_ln(0, e0)
            front_xy(0)
            issue_loads(2)
            for t in range(1, NT):
                front_a(t)
                gather(t, early=True)       # srcs in tiles <= t
                gather(t - 1, early=False)  # srcs in tile t (just computed)
                e = front_exp(t)
                front_ln(t, e)
                front_xy(t)
                back_chunk(t - 1, 0)
                back_chunk(t - 1, 1)
                back_chunk(t - 1, 2)
                issue_loads(t + 2)
            for c in range(3):
                back_chunk(NT - 1, c)

            nc.sync.dma_start(out=st_d[:, :], in_=stats)
    return nc


_NC_CACHE = None
LAST_RESULTS = None


def _ensure_ntff_hook():
    """Register the axon NTFF profiling hook (absent from this image's
    antenv) so trace=True can measure HW exec time. Trace-path only."""
    import contextlib
    import ctypes
    import types

    try:
        from antenv.axon_hooks import get_axon_ntff_profile_hook  # noqa: F401
        return
    except ImportError:
        pass

    holder = {}
    mod = types.ModuleType("antenv.axon_hooks")
    mod.set_axon_ntff_profile_hook = lambda h: holder.__setitem__("h", h)
    mod.get_axon_ntff_profile_hook = lambda: holder.get("h")
    sys.modules["antenv.axon_hooks"] = mod
    import antenv

    antenv.axon_hooks = mod

    so_path = "/opt/axon/libaxon_pjrt.so"
    try:
        lib = ctypes.CDLL(so_path)
    except OSError:
        return
    if not hasattr(lib, "axon_start_nrt_profile"):
        return
    lib.axon_start_nrt_profile.argtypes = [
        ctypes.POINTER(ctypes.c_int64), ctypes.c_size_t]
    lib.axon_start_nrt_profile.restype = ctypes.c_int64
    lib.axon_stop_nrt_profile.argtypes = [ctypes.c_char_p]
    lib.axon_stop_nrt_profile.restype = ctypes.c_int64

    @contextlib.contextmanager
    def _hook(output_dir, device_ids):
        import jax

        jax.devices()
        if device_ids:
            ids = (ctypes.c_int64 * len(device_ids))(*device_ids)
            rc = lib.axon_start_nrt_profile(ids, len(device_ids))
        else:
            rc = lib.axon_start_nrt_profile(None, 0)
        if rc != 0:
            raise RuntimeError(f"axon_start_nrt_profile rc={rc}")
        try:
            yield
        finally:
            n = lib.axon_stop_nrt_profile(str(output_dir).encode())
            print(f"profile: {n} file(s) written to {output_dir}",
                  file=sys.stderr)

    mod.set_axon_ntff_profile_hook(_hook)

    # keep trace artifacts local — no bucket in this container
    import concourse.bass_utils as bu

    bu.upload_artifacts = lambda tmpdir: tmpdir


def _get_nc():
    global _NC_CACHE
    if _NC_CACHE is None:
        _NC_CACHE = _build_nc()
    return _NC_CACHE


def _shard_inputs(x, y):
    """Host re-layout: per-core [row=(b*192+w), j, h] arrays."""
    xs, ys = [], []
    x5 = x[:, 0]  # [B, D, H, W]
    y5 = y[:, 0]
    for c in range(NCORES):
        dids = (np.arange(DSH * c - 1, DSH * (c + 1) + 1)) % D          # 26
        # [B, 26, H, W] -> [B, W, 26, H] -> [768, 26, 192]
        xs.append(np.ascontiguousarray(
            x5[:, dids].transpose(0, 3, 1, 2)).reshape(ROWS, J, H))
        ys.append(np.ascontiguousarray(
            y5[:, DSH * c:DSH * (c + 1)].transpose(0, 3, 1, 2)
        ).reshape(ROWS, DSH, H))
    return xs, ys


def kernel(x, y, is_gt):
    global LAST_RESULTS
    x = np.asarray(x, dtype=np.float32)
    y = np.asarray(y, dtype=np.float32)
    is_gt = np.asarray(is_gt, dtype=np.float32)

    nc = _get_nc()
    xs, ys = _shard_inputs(x, y)
    in_maps = [{"x": xs[c], "y": ys[c], "consts": _CONSTS}
               for c in range(NCORES)]
    trace = bool(os.environ.get("KERNEL_TRACE"))
    if trace:
        _ensure_ntff_hook()
    res = run_bass_kernel_spmd(nc, in_maps, core_ids=list(range(NCORES)),
                               trace=trace)
    LAST_RESULTS = res

    # host-side final reduction in fp64
    b_of = (np.arange(ROWS) // Wd)                       # global row -> sample
    S_xy = np.zeros(B)
    S_sp = np.zeros(B)
    S_cnt = np.zeros(B)
    for c in range(NCORES):
        st = res.results[c]["stats"].astype(np.float64)  # [128, NSTAT]
        for col in range(NSTAT):
            t, k = col // TCOLS, col % TCOLS
            tgt = S_xy if k < 1 else (S_sp if k < 2 else S_cnt)
            np.add.at(tgt, b_of[P * t:P * (t + 1)], st[:, col])
    bce = (S_sp - S_xy) / VOX
    conn = S_cnt / VOX
    loss = np.sum(is_gt.astype(np.float64) * bce + W_CONN * conn)
    return np.array([loss], dtype=np.float32)
